# revision 1
# baseline (speedup 1.0000x reference)
"""Trainium2 Bass kernel for a T5-style decoder layer (self-attn with causal
rel-pos bias, cross-attn, FFN, 3 post-LNs).

Sharding: 8 cores = (batch b in 0..3) x (parity g in 0..1). Core (b, g) owns
query blocks {g, 2+g, 4+g, 6+g} (128 rows each) of batch b and computes the
full layer for those 512 rows. K/V work is duplicated across the pair; no
cross-core collectives. Causal score work is padded to a uniform (2,4,6,8)
key-block pattern so one SPMD program serves all cores; padded blocks are
killed by the multiplicative bias table (EB = 0 there).

Key design points vs the f32r baseline:
- all matmul operands are bf16 (halves weight DMA, avoids the fp32r
  small-free-dim penalty); the residual stream stays f32.
- V is produced directly in [keys, d] layout by using the activation tile as
  matmul lhsT and the full weight row-block as rhs — no PE transposes and no
  per-head reassembly copies.
- rel-pos bias + causal mask enter as EB[h,i,w] = exp(band((w-128+128g)-i)),
  multiplied into exp(scores/8) on DVE (exp(a+b) = exp(a)exp(b)); the
  identity-matmul bias injection is gone.
- K bias dropped (softmax is invariant to a per-query shift); V bias folded
  into an effective output bias bo_eff = bo + bv @ wo on the host.
- softmax normalization and LN row broadcasts use tiny PE matmuls against
  ones rows; drains are split across ACT/DVE (Pool cannot touch PSUM or
  convert dtypes, so it only gets SBUF-resident f32 elementwise work).
- LN sum/sum-of-squares matmuls are fused into the producing output loops
  (one d-block behind the producer) so only the short row math remains at
  stage boundaries; the gain folds into ACT's per-partition scale.
- the output stays transposed [D, TOK]; the host transposes on unshard.
- cross-attention K/V projections are emitted before LN1 so PE has
  independent matmul work while DVE/ACT run the LayerNorm; DMA issue order
  is tuned so the first K matmul isn't queued behind bulky transfers.
"""

import functools
import math

import numpy as np
import ml_dtypes

import concourse.bass as bass
import concourse.bacc as bacc
import concourse.mybir as mybir
import concourse.tile as tile
from concourse.bass_utils import run_bass_kernel_spmd

F32 = mybir.dt.float32
F32R = mybir.dt.float32r
BF16 = mybir.dt.bfloat16
AL = mybir.AluOpType
AF = mybir.ActivationFunctionType
NPBF = ml_dtypes.bfloat16

B, L, D, H, DK, DFF = 4, 1024, 1024, 16, 64, 4096
P = 128
NB = D // P            # 8 d_model blocks
NF = DFF // P          # 32 d_ff blocks
TOK = 512              # tokens owned per core
SLOTS = 4              # query blocks of 128 per core
NUM_BUCKETS, MAX_DISTANCE = 32, 128
EPS = 1e-5


def _r(x):
    return x.bitcast(F32R)


def _f(x):
    return x.bitcast(F32)


def _build_nc(reps=1, dbg=False):
    nc = bacc.Bacc(trn_type="TRN2")

    def inp(name, shape, dt=BF16):
        return nc.declare_dram_parameter(name, list(shape), dt, isOutput=False)

    d_xo = inp("xoT", (D, TOK))            # own q slots, bf16, transposed
    d_xo32 = inp("xoT32", (D, TOK), F32)   # f32 copy for the residual
    d_xf = inp("xfT", (D, L))              # full x, bf16, transposed
    d_mm = inp("memT", (D, L))
    d_eb = inp("eb", (H, P, 1280))         # exp(bias band), mask zeros baked
    dw = {}
    for pre in ("sa", "ca"):
        for nm in ("wq", "wk", "wo"):
            dw[f"{pre}_{nm}"] = inp(f"{pre}_{nm}", (NB, P, NB, P))
        dw[f"{pre}_wv"] = inp(f"{pre}_wv", (P, NB, D))   # [p, t, dout]
        dw[f"{pre}_bq"] = inp(f"{pre}_bq", (P, NB), F32)
        dw[f"{pre}_bo"] = inp(f"{pre}_bo", (P, NB), F32)  # bo + bv @ wo
    d_fc1 = inp("fc1_w", (NF, P, NB, P))
    d_fc2 = inp("fc2_w", (NB, P, NF, P))
    d_fc1b = inp("fc1_b", (P, NF), F32)
    d_fc2b = inp("fc2_b", (P, NB), F32)
    dln = {}
    for i in ("1", "2", "3"):
        dln[f"g{i}"] = inp(f"ln{i}_g", (P, NB), F32)
        dln[f"b{i}"] = inp(f"ln{i}_b", (P, NB), F32)
    d_out = nc.declare_dram_parameter("out_ownT", [D, TOK], F32, isOutput=True)
    d_dbg = {}
    if dbg:
        for nm in ("sa_pre", "x1", "x2_pre", "x2"):
            d_dbg[nm] = nc.declare_dram_parameter(f"dbg_{nm}", [D, TOK], F32,
                                                  isOutput=True)
        d_dbg["ao"] = nc.declare_dram_parameter("dbg_ao", [D, TOK], BF16,
                                                isOutput=True)
        d_dbg["pt"] = nc.declare_dram_parameter("dbg_pt", [2 * P, TOK], BF16,
                                                isOutput=True)
        d_dbg["vn"] = nc.declare_dram_parameter("dbg_vn", [P, H, 65], BF16,
                                                isOutput=True)

    with (
        nc.allow_low_precision(reason="bf16 matmuls; f32 residual stream"),
        tile.TileContext(nc) as tc,
    ):
        with (
            tc.tile_pool(name="persist", bufs=1) as pers,
            tc.tile_pool(name="psum", bufs=1, space="PSUM") as psum,
        ):
            ones_src = pers.tile([P, P], F32, tag="onessrc")
            nc.gpsimd.memset(ones_src[:], 1.0)
            onesf = pers.tile([P, 1], F32R, tag="onesf")
            nc.scalar.copy(out=onesf[:], in_=ones_src[:, 0:1])
            ones_row = pers.tile([1, P], F32R, tag="onesr")
            nc.scalar.copy(out=ones_row[:], in_=ones_src[0:1, :])
            eps_t = pers.tile([1, 1], F32, tag="epsc")
            nc.gpsimd.memset(eps_t[:], EPS)

            for _rep in range(reps):
                # bias/LN tiles allocated now, DMAs issued after the SA
                # critical-path loads (bufs=2 so next rep's loads overlap).
                bias_sb = {}
                for k in ("sa_bq", "sa_bo", "ca_bq", "ca_bo"):
                    bias_sb[k] = pers.tile([P, NB], F32, tag=f"b_{k}",
                                           name=f"b_{k}", bufs=2)
                fc1b = pers.tile([P, NF], F32, tag="fc1b", bufs=2)
                fc2b = pers.tile([P, NB], F32, tag="fc2b", bufs=2)
                ln_sb = {}
                for k in dln:
                    ln_sb[k] = pers.tile([P, NB], F32, tag=f"ln_{k}",
                                         name=f"ln_{k}", bufs=2)

                def issue_small_dmas():
                    for k in ("sa_bq", "sa_bo", "ca_bq", "ca_bo"):
                        nc.sync.dma_start(out=bias_sb[k][:], in_=dw[k][:, :])
                    nc.sync.dma_start(out=fc1b[:], in_=d_fc1b[:, :])
                    nc.sync.dma_start(out=fc2b[:], in_=d_fc2b[:, :])
                    for k, dv in dln.items():
                        nc.sync.dma_start(out=ln_sb[k][:], in_=dv[:, :])

                def ln_sums_alloc():
                    pm = psum.tile([1, TOK], F32, tag="plnA", name="pm",
                                   bufs=1)
                    pv2 = psum.tile([1, TOK], F32, tag="plnB", name="pv2",
                                    bufs=1)
                    return pm, pv2

                def ln_sums_step(pm, pv2, src_d, d, pool):
                    nc.tensor.matmul(pm[:], _r(onesf[:]), _r(src_d[:]),
                                     start=(d == 0), stop=(d == NB - 1))
                    sq = pool.tile([P, TOK], F32R, tag="lnsq", bufs=2)
                    if d % 2 == 0:
                        nc.scalar.square(sq[:], _f(src_d[:]))
                    else:
                        nc.gpsimd.tensor_mul(sq[:], _f(src_d[:]),
                                             _f(src_d[:]))
                    nc.tensor.matmul(pv2[:], _r(onesf[:]), _r(sq[:]),
                                     start=(d == 0), stop=(d == NB - 1))

                def layernorm_T(src, g_ap, b_ap, out_f32, out_b16,
                                pool, sums=None):
                    """LN over partitions (d) of src (f32 [P,TOK] x NB).
                    Writes f32 out_f32 and (optionally) bf16 out_b16."""
                    if sums is None:
                        pm, pv2 = ln_sums_alloc()
                        for d in range(NB):
                            ln_sums_step(pm, pv2, src[d], d, pool)
                    else:
                        pm, pv2 = sums
                    mu = pool.tile([1, TOK], F32R, tag="lnmu")
                    with nc.allow_low_precision(reason="f32r mu row"):
                        nc.vector.tensor_scalar_mul(mu[:], pm[:], 1.0 / D)
                    musq = pool.tile([1, TOK], F32, tag="lnmusq")
                    nc.vector.tensor_mul(musq[:], _f(mu[:]), _f(mu[:]))
                    var = pool.tile([1, TOK], F32, tag="lnvar")
                    nc.vector.scalar_tensor_tensor(
                        var[:], pv2[:], 1.0 / D, musq[:],
                        op0=AL.mult, op1=AL.subtract)
                    std = pool.tile([1, TOK], F32, tag="lnstd")
                    nc.scalar.activation(std[:], var[:], AF.Sqrt,
                                         bias=eps_t[:])
                    rsd = pool.tile([1, TOK], F32R, tag="lnrsd")
                    with nc.allow_low_precision(reason="f32r rsd row"):
                        nc.vector.reciprocal(rsd[:], std[:])
                    mu_r = pool.tile([P, TOK], F32, tag="lnmur")
                    rsd_r = pool.tile([P, TOK], F32, tag="lnrsdr")
                    for row, rep in ((mu, mu_r), (rsd, rsd_r)):
                        pb = psum.tile([P, TOK], F32, tag="pgen", name="pbc",
                                       bufs=1)
                        nc.tensor.matmul(pb[:], _r(ones_row[:]), _r(row[:]))
                        nc.scalar.copy(out=rep[:], in_=pb[:])
                    for d in range(NB):
                        eng = nc.vector if d % 2 == 0 else nc.gpsimd
                        t1 = pool.tile([P, TOK], F32, tag="lnt1", bufs=4)
                        eng.tensor_tensor(out=t1[:], in0=_f(src[d][:]),
                                          in1=mu_r[:], op=AL.subtract)
                        t2 = pool.tile([P, TOK], F32, tag="lnt2", bufs=4)
                        eng.tensor_tensor(out=t2[:], in0=t1[:], in1=rsd_r[:],
                                          op=AL.mult)
                        # gain folds into ACT's per-partition scale
                        nc.scalar.activation(out_f32[d][:], t2[:], AF.Identity,
                                             bias=b_ap[:, d:d + 1],
                                             scale=g_ap[:, d:d + 1])
                        if out_b16 is not None:
                            if d % 2 == 0:
                                nc.vector.tensor_copy(out_b16[d][:],
                                                      _f(out_f32[d][:]))
                            else:
                                nc.scalar.copy(out=out_b16[d][:],
                                               in_=_f(out_f32[d][:]))

                def attn_kv(tc_, pre, kvT, pool, pending_dmas=(),
                            mid_emit=None):
                    """K projections + direct-layout V for all heads.
                    Returns (k_sb[hp] bf16 [P,L], vn[kb] bf16 [P,H,65]).
                    K runs first (scores need it sooner); both K and V^T
                    work in 512-wide halves on the shared "ps" PSUM ring so
                    drain copies double-buffer against the next matmul.
                    DMA issue order: wk0 before the bulky wv transfer so the
                    first K matmul is not stuck behind it in the DGE queue;
                    pending_dmas (input tails) issue in between."""
                    k_sb = []
                    vn = []
                    with tc_.tile_pool(name=f"{pre}_wv", bufs=1) as wvp:
                        wv_t = wvp.tile([P, NB, D], BF16, tag="wvt", bufs=1)
                        for hp in range(NB):
                            wk_t = pool.tile([P, NB, P], BF16, tag="wkt",
                                             bufs=3)
                            nc.sync.dma_start(out=wk_t[:],
                                              in_=dw[f"{pre}_wk"][hp])
                            if hp == 0:
                                for fn in pending_dmas:
                                    fn()
                            if hp == 4:
                                nc.sync.dma_start(
                                    out=wv_t[:], in_=dw[f"{pre}_wv"][:, :, :])
                            k = pool.tile([P, L], BF16, tag=f"ks{hp}",
                                          name=f"ks{hp}", bufs=1)
                            for half in range(2):
                                sl = slice(half * 512, (half + 1) * 512)
                                pkh = psum.tile([P, 512], F32, tag="ps",
                                                name="pkh", bufs=3)
                                for t in range(NB):
                                    nc.tensor.matmul(pkh[:], wk_t[:, t, :],
                                                     kvT[t][:, sl],
                                                     start=(t == 0),
                                                     stop=(t == NB - 1))
                                nc.scalar.copy(out=k[:, sl], in_=pkh[:])
                            k_sb.append(k)
                        if mid_emit is not None:
                            mid_emit()
                        for kb in range(NB):
                            v = pool.tile([P, H, 65], BF16, tag=f"vn{kb}",
                                          name=f"vn{kb}", bufs=1)
                            nc.gpsimd.memset(v[:, :, 64:65], 1.0)
                            for half in range(2):
                                sl = slice(half * 512, (half + 1) * 512)
                                pvh = psum.tile([P, 512], F32, tag="ps",
                                                name="pvh", bufs=3)
                                for t in range(NB):
                                    nc.tensor.matmul(
                                        pvh[:], kvT[t][:, kb * P:(kb + 1) * P],
                                        wv_t[:, t, sl],
                                        start=(t == 0), stop=(t == NB - 1))
                                nc.scalar.copy(
                                    out=v[:, half * 8:(half + 1) * 8, 0:64],
                                    in_=pvh[:].rearrange("p (h c) -> p h c",
                                                         c=64))
                            vn.append(v)
                    return k_sb, vn

                def attn_q(pre, q_src, k_sb, vn, causal, out_tiles, resid,
                           pool, post_db=None):
                    """Q proj + scores + softmax + AV + O proj (+resid)."""
                    bq = bias_sb[f"{pre}_bq"]
                    bo = bias_sb[f"{pre}_bo"]
                    AO = [pool.tile([P, TOK], BF16, tag=f"ao{hp}",
                                    name=f"ao{hp}", bufs=1)
                          for hp in range(NB)]
                    wo_pre = {}
                    for db in range(3):
                        w = pool.tile([P, NB, P], BF16, tag="wot", bufs=3)
                        nc.sync.dma_start(out=w[:], in_=dw[f"{pre}_wo"][db])
                        wo_pre[db] = w
                    for hp in range(NB):
                        wq_t = pool.tile([P, NB, P], BF16, tag="wqt", bufs=3)
                        nc.sync.dma_start(out=wq_t[:], in_=dw[f"{pre}_wq"][hp])
                        pq = psum.tile([P, TOK], F32, tag="pgen", name="pq",
                                       bufs=1)
                        for t in range(NB):
                            nc.tensor.matmul(pq[:], wq_t[:, t, :],
                                             q_src[t][:],
                                             start=(t == 0), stop=(t == NB - 1))
                        q_sb = pool.tile([P, TOK], BF16, tag="qsb", bufs=2)
                        if causal:
                            nc.scalar.activation(q_sb[:], pq[:], AF.Identity,
                                                 bias=bq[:, hp:hp + 1],
                                                 scale=1.0)
                        else:
                            nc.vector.tensor_scalar_add(q_sb[:], pq[:],
                                                        bq[:, hp:hp + 1])

                        hsls = (slice(0, 64), slice(64, 128))
                        eb_ts = []
                        if causal:
                            for hh in range(2):
                                eb_t = pool.tile([P, 1280], BF16, tag="ebt",
                                                 bufs=3)
                                nc.sync.dma_start(out=eb_t[:],
                                                  in_=d_eb[2 * hp + hh])
                                eb_ts.append(eb_t)
                        # interleave the two heads of this partition block so
                        # PE always has a score matmul ready while ACT exps.
                        pts = ([], [])
                        for kb in range(NB):
                            smin = kb // 2 if causal else 0
                            n = TOK - smin * P
                            ns = SLOTS - smin
                            for hh in range(2):
                                ps = psum.tile([P, TOK], F32, tag="ps",
                                               name="ps", bufs=3)
                                nc.tensor.matmul(
                                    ps[:, 0:n],
                                    k_sb[hp][hsls[hh], kb * P:(kb + 1) * P],
                                    q_sb[hsls[hh], smin * P:TOK],
                                    start=True, stop=True)
                                pt = pool.tile([P, TOK], BF16, tag="pt",
                                               bufs=8)
                                nc.scalar.activation(pt[:, 0:n], ps[:, 0:n],
                                                     AF.Exp, scale=0.125)
                                if causal:
                                    w0 = 256 * smin - 128 * kb + 128
                                    ebv = eb_ts[hh][:, w0:w0 + ns * 256] \
                                        .rearrange("p (s c) -> p s c",
                                                   c=256)[:, :, 0:P]
                                    nc.vector.tensor_tensor(
                                        out=pt[:, 0:n].rearrange(
                                            "p (s c) -> p s c", c=P),
                                        in0=pt[:, 0:n].rearrange(
                                            "p (s c) -> p s c", c=P),
                                        in1=ebv, op=AL.mult)
                                pts[hh].append(pt)
                        if dbg and pre == "sa" and hp == 0:
                            nc.sync.dma_start(out=d_dbg["vn"][:, :, :],
                                              in_=vn[0][:])
                            for kb_ in range(2):
                                nc.sync.dma_start(
                                    out=d_dbg["pt"][kb_ * P:(kb_ + 1) * P, :],
                                    in_=pts[0][kb_][:])
                        for hh in range(2):
                            h = 2 * hp + hh
                            pav = psum.tile([65, TOK], F32, tag="pav",
                                            name="pav", bufs=2)
                            for kb in range(NB):
                                smin = kb // 2 if causal else 0
                                n = TOK - smin * P
                                nc.tensor.matmul(
                                    pav[:, smin * P:TOK], vn[kb][:, h, :],
                                    pts[hh][kb][:, 0:n],
                                    start=(kb == 0), stop=(kb == NB - 1))
                            rec = pool.tile([1, TOK], F32R, tag="rec",
                                            bufs=2)
                            with nc.allow_low_precision(reason="recip row"):
                                nc.vector.reciprocal(rec[:], pav[64:65, :])
                            prr = psum.tile([64, TOK], F32, tag="ps",
                                            name="prr", bufs=3)
                            nc.tensor.matmul(prr[:], _r(ones_row[0:1, 0:64]),
                                             _r(rec[:]))
                            rrep = pool.tile([64, TOK], F32, tag="rrep",
                                             bufs=1)
                            nc.vector.tensor_copy(rrep[:], prr[:])
                            nc.vector.tensor_tensor(
                                out=AO[hp][hsls[hh], :], in0=pav[0:64, :],
                                in1=rrep[:], op=AL.mult)

                    if dbg and pre == "sa":
                        for hp in range(NB):
                            nc.sync.dma_start(
                                out=d_dbg["ao"][hp * P:(hp + 1) * P, :],
                                in_=AO[hp][:])
                    for db in range(NB):
                        if db in wo_pre:
                            wo_t = wo_pre.pop(db)
                        else:
                            wo_t = pool.tile([P, NB, P], BF16, tag="wot",
                                             bufs=3)
                            nc.sync.dma_start(out=wo_t[:],
                                              in_=dw[f"{pre}_wo"][db])
                        po = psum.tile([P, TOK], F32, tag="pgen", name="po",
                                       bufs=1)
                        for hp in range(NB):
                            nc.tensor.matmul(po[:], wo_t[:, hp, :],
                                             AO[hp][:],
                                             start=(hp == 0),
                                             stop=(hp == NB - 1))
                        nc.vector.scalar_tensor_tensor(
                            out_tiles[db][:], po[:], bo[:, db:db + 1],
                            _f(resid[db][:]), op0=AL.add, op1=AL.add)
                        if post_db is not None and db > 0:
                            post_db(db - 1, out_tiles[db - 1])
                    if post_db is not None:
                        post_db(NB - 1, out_tiles[NB - 1])

                with tc.tile_pool(name="xs", bufs=1) as xsp:
                    x1 = [xsp.tile([P, TOK], F32R, tag=f"x1_{d}",
                                   name=f"x1_{d}") for d in range(NB)]
                    x1b = [xsp.tile([P, TOK], BF16, tag=f"x1b_{d}",
                                    name=f"x1b_{d}") for d in range(NB)]
                    x2 = [xsp.tile([P, TOK], F32R, tag=f"x2_{d}",
                                   name=f"x2_{d}") for d in range(NB)]
                    x2b = [xsp.tile([P, TOK], BF16, tag=f"x2b_{d}",
                                    name=f"x2b_{d}") for d in range(NB)]
                    mm = [xsp.tile([P, L], BF16, tag=f"mm{d}",
                                   name=f"mm{d}") for d in range(NB)]

                    # ---------------- self-attention ----------------
                    with tc.tile_pool(name="sa", bufs=1) as sa_pool:
                        xf = [sa_pool.tile([P, L], BF16, tag=f"xf{d}",
                                           name=f"xf{d}") for d in range(NB)]
                        for d in range(2):
                            nc.sync.dma_start(
                                out=xf[d][:, 0:512],
                                in_=d_xf[d * P:(d + 1) * P, 0:512])
                        xo = [sa_pool.tile([P, TOK], BF16, tag=f"xo{d}",
                                           name=f"xo{d}") for d in range(NB)]
                        xo32 = [sa_pool.tile([P, TOK], F32, tag=f"xo32{d}",
                                             name=f"xo32{d}")
                                for d in range(NB)]

                        def _xf_dma(d, half):
                            sl = slice(half * 512, (half + 1) * 512)
                            return lambda: nc.sync.dma_start(
                                out=xf[d][:, sl],
                                in_=d_xf[d * P:(d + 1) * P, sl])

                        ln1_sums = ln_sums_alloc()

                        def _ln1_step(db, tile_):
                            ln_sums_step(ln1_sums[0], ln1_sums[1], tile_,
                                         db, sa_pool)

                        pending = [_xf_dma(d, 0) for d in range(2, NB)] + \
                            [_xf_dma(d, 1) for d in range(NB)]
                        k_sb, vn = attn_kv(tc, "sa", xf, sa_pool, pending)
                        for d in range(NB):
                            nc.sync.dma_start(
                                out=xo[d][:], in_=d_xo[d * P:(d + 1) * P, :])
                            nc.sync.dma_start(
                                out=xo32[d][:],
                                in_=d_xo32[d * P:(d + 1) * P, :])
                        for d in range(NB):
                            nc.sync.dma_start(
                                out=mm[d][:], in_=d_mm[d * P:(d + 1) * P, :])
                        issue_small_dmas()
                        attn_q("sa", xo, k_sb, vn, True, x1, xo32,
                               sa_pool, post_db=_ln1_step)
                        if dbg:
                            for d in range(NB):
                                nc.sync.dma_start(
                                    out=d_dbg["sa_pre"][d * P:(d + 1) * P, :],
                                    in_=_f(x1[d][:]))

                    # ---------------- cross-attention ----------------
                    # CA K/V first: PE chews mem-dependent matmuls while
                    # DVE/ACT run LN1.
                    with tc.tile_pool(name="ca", bufs=1) as ca_pool:
                        def _ln1_finish():
                            layernorm_T(x1, ln_sb["g1"][:], ln_sb["b1"][:],
                                        x1, x1b, ca_pool, sums=ln1_sums)

                        ck_sb, cvn = attn_kv(tc, "ca", mm, ca_pool,
                                             mid_emit=_ln1_finish)
                        if dbg:
                            for d in range(NB):
                                nc.sync.dma_start(
                                    out=d_dbg["x1"][d * P:(d + 1) * P, :],
                                    in_=_f(x1[d][:]))
                        ln2_sums = ln_sums_alloc()

                        def _ln2_step(db, tile_):
                            ln_sums_step(ln2_sums[0], ln2_sums[1], tile_,
                                         db, ca_pool)

                        attn_q("ca", x1b, ck_sb, cvn, False, x2, x1, ca_pool,
                               post_db=_ln2_step)
                        if dbg:
                            for d in range(NB):
                                nc.sync.dma_start(
                                    out=d_dbg["x2_pre"][d * P:(d + 1) * P, :],
                                    in_=_f(x2[d][:]))

                    # ---------------- FFN ----------------
                    with tc.tile_pool(name="ff", bufs=1) as ff_pool:
                        w1_pre = {}
                        for ff in range(3):
                            w = ff_pool.tile([P, NB, P], BF16, tag="w1t",
                                             bufs=3)
                            nc.sync.dma_start(out=w[:], in_=d_fc1[ff])
                            w1_pre[ff] = w
                        layernorm_T(x2, ln_sb["g2"][:], ln_sb["b2"][:],
                                    x2, x2b, ff_pool, sums=ln2_sums)
                        if dbg:
                            for d in range(NB):
                                nc.sync.dma_start(
                                    out=d_dbg["x2"][d * P:(d + 1) * P, :],
                                    in_=_f(x2[d][:]))
                        ht = []
                        w2_pre = {}
                        for ff in range(NF):
                            if ff in w1_pre:
                                w1 = w1_pre.pop(ff)
                            else:
                                w1 = ff_pool.tile([P, NB, P], BF16, tag="w1t",
                                                  bufs=3)
                                nc.sync.dma_start(out=w1[:], in_=d_fc1[ff])
                            if ff in (16, 24):
                                db_ = (ff - 16) // 8
                                w = ff_pool.tile([P, NF, P], BF16, tag="w2t",
                                                 bufs=2)
                                nc.sync.dma_start(out=w[:], in_=d_fc2[db_])
                                w2_pre[db_] = w
                            pf = psum.tile([P, TOK], F32, tag="pgen",
                                           name="pf", bufs=1)
                            for t in range(NB):
                                nc.tensor.matmul(pf[:], w1[:, t, :],
                                                 x2b[t][:],
                                                 start=(t == 0),
                                                 stop=(t == NB - 1))
                            h = ff_pool.tile([P, TOK], BF16, tag=f"ht{ff}",
                                             name=f"ht{ff}")
                            nc.scalar.activation(h[:], pf[:], AF.Relu,
                                                 bias=fc1b[:, ff:ff + 1],
                                                 scale=1.0)
                            ht.append(h)
                        x3 = [ff_pool.tile([P, TOK], F32R, tag=f"x3_{d}",
                                           name=f"x3_{d}")
                              for d in range(NB)]
                        ln3_sums = ln_sums_alloc()
                        for db in range(NB):
                            if db in w2_pre:
                                w2 = w2_pre.pop(db)
                            else:
                                w2 = ff_pool.tile([P, NF, P], BF16, tag="w2t",
                                                  bufs=2)
                                nc.sync.dma_start(out=w2[:], in_=d_fc2[db])
                            pf2 = psum.tile([P, TOK], F32, tag="ps",
                                            name="pf2", bufs=3)
                            for t in range(NF):
                                nc.tensor.matmul(pf2[:], w2[:, t, :],
                                                 ht[t][:],
                                                 start=(t == 0),
                                                 stop=(t == NF - 1))
                            nc.vector.scalar_tensor_tensor(
                                x3[db][:], pf2[:], fc2b[:, db:db + 1],
                                _f(x2[db][:]), op0=AL.add, op1=AL.add)
                            if db > 0:
                                ln_sums_step(ln3_sums[0], ln3_sums[1],
                                             x3[db - 1], db - 1, ff_pool)
                        ln_sums_step(ln3_sums[0], ln3_sums[1], x3[NB - 1],
                                     NB - 1, ff_pool)
                        x3ln = [ff_pool.tile([P, TOK], F32, tag=f"x3l{d}",
                                             name=f"x3l{d}")
                                for d in range(NB)]
                        layernorm_T(x3, ln_sb["g3"][:], ln_sb["b3"][:],
                                    x3ln, None, ff_pool, sums=ln3_sums)
                        for db in range(NB):
                            nc.sync.dma_start(
                                out=d_out[db * P:(db + 1) * P, :],
                                in_=x3ln[db][:])

    nc.finalize()
    return nc


@functools.lru_cache(maxsize=4)
def _get_nc(reps=1, dbg=False):
    return _build_nc(reps, dbg)


def _rel_bucket_np(v):
    """T5 causal bucket for relative distance v = q - k (>= 0)."""
    n = np.maximum(v, 0)
    max_exact = NUM_BUCKETS // 2
    nf = np.maximum(n.astype(np.float32), 1.0)
    val_large = max_exact + (
        np.log(nf / max_exact) / math.log(MAX_DISTANCE / max_exact)
        * (NUM_BUCKETS - max_exact)
    ).astype(np.int32)
    val_large = np.minimum(val_large, NUM_BUCKETS - 1)
    return np.where(n < max_exact, n, val_large).astype(np.int32)


def _build_eb(rel_emb, g):
    """EB[h, i, w] = exp(band((w - 128 + 128 g) - i)); 0 where q < k."""
    v = (np.arange(1280)[None, :] - 128 + 128 * g) - np.arange(P)[:, None]
    bucket = _rel_bucket_np(v)                      # [128, 1280]
    band = np.exp(rel_emb[bucket])                  # [128, 1280, 16]
    band = np.transpose(band, (2, 0, 1)).copy()     # [16, 128, 1280]
    band[:, v < 0] = 0.0
    return band.astype(NPBF)


def _rearr_bias(b):
    return np.ascontiguousarray(np.asarray(b).reshape(-1, P).T, np.float32)


def _tile4(w):
    kb, mb = w.shape[0] // P, w.shape[1] // P
    return np.ascontiguousarray(
        w.reshape(kb, P, mb, P).transpose(2, 1, 0, 3).astype(NPBF))


def _make_in_maps(inp):
    x = np.asarray(inp["x"], np.float32)
    mem = np.asarray(inp["mem"], np.float32)
    rel_emb = np.asarray(inp["rel_emb"], np.float32)

    shared = {}
    for k in ("sa_wq", "sa_wk", "sa_wo", "ca_wq", "ca_wk", "ca_wo",
              "fc1_w", "fc2_w"):
        shared[k] = _tile4(np.asarray(inp[k], np.float32))
    for pre in ("sa", "ca"):
        wv = np.asarray(inp[f"{pre}_wv"], np.float32)
        shared[f"{pre}_wv"] = np.ascontiguousarray(
            wv.reshape(NB, P, D).transpose(1, 0, 2).astype(NPBF))
        bo_eff = np.asarray(inp[f"{pre}_bo"], np.float32) + \
            np.asarray(inp[f"{pre}_bv"], np.float32) @ \
            np.asarray(inp[f"{pre}_wo"], np.float32)
        shared[f"{pre}_bo"] = _rearr_bias(bo_eff)
        shared[f"{pre}_bq"] = _rearr_bias(inp[f"{pre}_bq"])
    for k in ("fc1_b", "fc2_b", "ln1_g", "ln1_b", "ln2_g", "ln2_b",
              "ln3_g", "ln3_b"):
        shared[k] = _rearr_bias(inp[k])
    eb = [_build_eb(rel_emb, g) for g in range(2)]

    in_maps = []
    for c in range(8):
        b, g = c // 2, c % 2
        rows = np.concatenate(
            [x[b, (2 * s + g) * P:(2 * s + g + 1) * P] for s in range(SLOTS)])
        m = dict(shared)
        m["xoT"] = np.ascontiguousarray(rows.T.astype(NPBF))
        m["xoT32"] = np.ascontiguousarray(rows.T, np.float32)
        m["xfT"] = np.ascontiguousarray(x[b].T.astype(NPBF))
        m["memT"] = np.ascontiguousarray(mem[b].T.astype(NPBF))
        m["eb"] = eb[g]
        in_maps.append(m)
    return in_maps


def _unshard(results):
    out = np.empty((B, L, D), np.float32)
    for c in range(8):
        b, g = c // 2, c % 2
        oc = results[c]["out_ownT"]        # [D, TOK]
        for s in range(SLOTS):
            out[b, (2 * s + g) * P:(2 * s + g + 1) * P] = \
                oc[:, s * P:(s + 1) * P].T
    return out


def kernel(x, mem, tgt_mask, mem_mask,
           sa_wq, sa_bq, sa_wk, sa_bk, sa_wv, sa_bv, sa_wo, sa_bo, rel_emb,
           ca_wq, ca_bq, ca_wk, ca_bk, ca_wv, ca_bv, ca_wo, ca_bo,
           fc1_w, fc1_b, fc2_w, fc2_b,
           ln1_g, ln1_b, ln2_g, ln2_b, ln3_g, ln3_b, _trace=False):
    nc = _get_nc()
    in_maps = _make_in_maps(dict(
        x=x, mem=mem, rel_emb=rel_emb,
        sa_wq=sa_wq, sa_wk=sa_wk, sa_wv=sa_wv, sa_wo=sa_wo,
        sa_bq=sa_bq, sa_bk=sa_bk, sa_bv=sa_bv, sa_bo=sa_bo,
        ca_wq=ca_wq, ca_wk=ca_wk, ca_wv=ca_wv, ca_wo=ca_wo,
        ca_bq=ca_bq, ca_bk=ca_bk, ca_bv=ca_bv, ca_bo=ca_bo,
        fc1_w=fc1_w, fc1_b=fc1_b, fc2_w=fc2_w, fc2_b=fc2_b,
        ln1_g=ln1_g, ln1_b=ln1_b, ln2_g=ln2_g, ln2_b=ln2_b,
        ln3_g=ln3_g, ln3_b=ln3_b))

    res = run_bass_kernel_spmd(nc, in_maps, list(range(8)), trace=_trace)
    out = _unshard(res.results)
    kernel.last_exec_time_ns = res.exec_time_ns
    return out



# revision 17
# speedup vs baseline: 1.0198x; 1.0198x over previous
"""Trainium2 Bass kernel for a T5-style decoder layer (self-attn with causal
rel-pos bias, cross-attn, FFN, 3 post-LNs).

Sharding: 8 cores = (batch b in 0..3) x (parity g in 0..1). Core (b, g) owns
query blocks {g, 2+g, 4+g, 6+g} (128 rows each) of batch b and computes the
full layer for those 512 rows. K/V work is duplicated across the pair; no
cross-core collectives. Causal score work is padded to a uniform (2,4,6,8)
key-block pattern so one SPMD program serves all cores; padded blocks are
killed by the multiplicative bias table (EB = 0 there).

Key design points vs the f32r baseline:
- all matmul operands are bf16 (halves weight DMA, avoids the fp32r
  small-free-dim penalty); the residual stream stays f32.
- V is produced directly in [keys, d] layout by using the activation tile as
  matmul lhsT and the full weight row-block as rhs — no PE transposes and no
  per-head reassembly copies.
- rel-pos bias + causal mask enter as EB[h,i,w] = exp(band((w-128+128g)-i)),
  multiplied into exp(scores/8) on DVE (exp(a+b) = exp(a)exp(b)); the
  identity-matmul bias injection is gone.
- K bias dropped (softmax is invariant to a per-query shift); V bias folded
  into an effective output bias bo_eff = bo + bv @ wo on the host.
- softmax normalization and LN row broadcasts use tiny PE matmuls against
  ones rows; drains are split across ACT/DVE (Pool cannot touch PSUM or
  convert dtypes, so it only gets SBUF-resident f32 elementwise work).
- LN sum/sum-of-squares matmuls are fused into the producing output loops
  (one d-block behind the producer) so only the short row math remains at
  stage boundaries; the gain folds into ACT's per-partition scale.
- the output stays transposed [D, TOK]; the host transposes on unshard.
- cross-attention K/V projections are emitted before LN1 so PE has
  independent matmul work while DVE/ACT run the LayerNorm; DMA issue order
  is tuned so the first K matmul isn't queued behind bulky transfers.
"""

import functools
import math

import numpy as np
import ml_dtypes

import concourse.bass as bass
import concourse.bacc as bacc
import concourse.mybir as mybir
import concourse.tile as tile
from concourse.bass_utils import run_bass_kernel_spmd

F32 = mybir.dt.float32
F32R = mybir.dt.float32r
BF16 = mybir.dt.bfloat16
AL = mybir.AluOpType
AF = mybir.ActivationFunctionType
NPBF = ml_dtypes.bfloat16

B, L, D, H, DK, DFF = 4, 1024, 1024, 16, 64, 4096
P = 128
NB = D // P            # 8 d_model blocks
NF = DFF // P          # 32 d_ff blocks
TOK = 512              # tokens owned per core
SLOTS = 4              # query blocks of 128 per core
NUM_BUCKETS, MAX_DISTANCE = 32, 128
EPS = 1e-5
STRIP_LDW = False


def _r(x):
    return x.bitcast(F32R)


def _f(x):
    return x.bitcast(F32)


def _strip_redundant_ldweights(nc):
    """Drop an InstLdweights whose weights (same SBUF AP+offset) are already
    resident in the PE array from the immediately preceding load, and which
    carries no semaphore waits/updates. The legalizer emits one load per
    matmul even for back-to-back weight-stationary pairs; the reload is pure
    PE-sequencer/array time on hardware."""
    removed = 0
    for blk in nc.m.functions[0].blocks:
        insts = list(blk.instructions)
        last_w = None
        keep = []
        for inst in insts:
            nm = type(inst).__name__
            if nm == "InstLdweights":
                w = inst.ins[0]
                key = (str(w.ap.to_list()), getattr(w, "offset", None))
                si = inst.sync_info
                clean = si is None or (len(si.on_wait) == 0
                                       and len(si.on_update) == 0)
                if clean and key == last_w:
                    removed += 1
                    continue
                last_w = key
            keep.append(inst)
        if len(keep) != len(insts):
            blk.instructions = keep
    return removed


def _build_nc(reps=1, dbg=False):
    nc = bacc.Bacc(trn_type="TRN2")

    def inp(name, shape, dt=BF16):
        return nc.declare_dram_parameter(name, list(shape), dt, isOutput=False)

    d_xo = inp("xoT", (D, TOK))            # own q slots, bf16, transposed
    d_xo32 = inp("xoT32", (D, TOK), F32)   # f32 copy for the residual
    d_xf = inp("xfT", (D, L))              # full x, bf16, transposed
    d_mm = inp("memT", (D, L))
    d_eb = inp("eb", (H, P, 1280))         # exp(bias band), mask zeros baked
    dw = {}
    for pre in ("sa", "ca"):
        for nm in ("wq", "wk", "wo"):
            dw[f"{pre}_{nm}"] = inp(f"{pre}_{nm}", (NB, P, NB, P))
        dw[f"{pre}_wv"] = inp(f"{pre}_wv", (P, NB, D))   # [p, t, dout]
        dw[f"{pre}_bq"] = inp(f"{pre}_bq", (P, NB), F32)
        dw[f"{pre}_bo"] = inp(f"{pre}_bo", (P, NB), F32)  # bo + bv @ wo
    d_fc1 = inp("fc1_w", (NF, P, NB, P))
    d_fc2 = inp("fc2_w", (NB, P, NF, P))
    d_fc1b = inp("fc1_b", (P, NF), F32)
    d_fc2b = inp("fc2_b", (P, NB), F32)
    dln = {}
    for i in ("1", "2", "3"):
        dln[f"g{i}"] = inp(f"ln{i}_g", (P, NB), F32)
        dln[f"b{i}"] = inp(f"ln{i}_b", (P, NB), F32)
    d_out = nc.declare_dram_parameter("out_ownT", [D, TOK], F32, isOutput=True)
    d_dbg = {}
    if dbg:
        for nm in ("sa_pre", "x1", "x2_pre", "x2"):
            d_dbg[nm] = nc.declare_dram_parameter(f"dbg_{nm}", [D, TOK], F32,
                                                  isOutput=True)
        d_dbg["ao"] = nc.declare_dram_parameter("dbg_ao", [D, TOK], BF16,
                                                isOutput=True)
        d_dbg["pt"] = nc.declare_dram_parameter("dbg_pt", [2 * P, TOK], BF16,
                                                isOutput=True)
        d_dbg["vn"] = nc.declare_dram_parameter("dbg_vn", [P, H, 65], BF16,
                                                isOutput=True)

    with (
        nc.allow_low_precision(reason="bf16 matmuls; f32 residual stream"),
        tile.TileContext(nc) as tc,
    ):
        with (
            tc.tile_pool(name="persist", bufs=1) as pers,
            tc.tile_pool(name="psum", bufs=1, space="PSUM") as psum,
        ):
            ones_src = pers.tile([P, P], F32, tag="onessrc")
            nc.gpsimd.memset(ones_src[:], 1.0)
            onesf = pers.tile([P, 1], F32R, tag="onesf")
            nc.scalar.copy(out=onesf[:], in_=ones_src[:, 0:1])
            eps_t = pers.tile([1, 1], F32, tag="epsc")
            nc.gpsimd.memset(eps_t[:], EPS)

            for _rep in range(reps):
                # bias/LN tiles allocated now, DMAs issued after the SA
                # critical-path loads (bufs=2 so next rep's loads overlap).
                bias_sb = {}
                for k in ("sa_bq", "sa_bo", "ca_bq", "ca_bo"):
                    bias_sb[k] = pers.tile([P, NB], F32, tag=f"b_{k}",
                                           name=f"b_{k}", bufs=2)
                fc1b = pers.tile([P, NF], F32, tag="fc1b", bufs=2)
                fc2b = pers.tile([P, NB], F32, tag="fc2b", bufs=2)
                ln_sb = {}
                for k in dln:
                    ln_sb[k] = pers.tile([P, NB], F32, tag=f"ln_{k}",
                                         name=f"ln_{k}", bufs=2)

                def issue_small_dmas():
                    for k in ("sa_bq", "sa_bo", "ca_bq", "ca_bo"):
                        nc.sync.dma_start(out=bias_sb[k][:], in_=dw[k][:, :])
                    nc.sync.dma_start(out=fc1b[:], in_=d_fc1b[:, :])
                    nc.sync.dma_start(out=fc2b[:], in_=d_fc2b[:, :])
                    for k, dv in dln.items():
                        nc.sync.dma_start(out=ln_sb[k][:], in_=dv[:, :])

                def ln_sums_alloc():
                    pm = psum.tile([1, TOK], F32, tag="plnA", name="pm",
                                   bufs=1)
                    pv2 = psum.tile([1, TOK], F32, tag="plnB", name="pv2",
                                    bufs=1)
                    return pm[:], pv2[:]

                def ln_sums_step(pm, pv2, src_d, d, pool):
                    nc.tensor.matmul(pm, _r(onesf[:]), _r(src_d[:]),
                                     start=(d == 0), stop=(d == NB - 1))
                    sq = pool.tile([P, TOK], F32R, tag="lnsq", bufs=2)
                    if d % 2 == 0:
                        nc.scalar.square(sq[:], _f(src_d[:]))
                    else:
                        nc.gpsimd.tensor_mul(sq[:], _f(src_d[:]),
                                             _f(src_d[:]))
                    nc.tensor.matmul(pv2, _r(onesf[:]), _r(sq[:]),
                                     start=(d == 0), stop=(d == NB - 1))

                def layernorm_T(src, g_ap, b_ap, out_f32, out_b16,
                                pool, sums=None, post_d=None):
                    """LN over partitions (d) of src (f32 [P,TOK] x NB).
                    Writes f32 out_f32 and (optionally) bf16 out_b16."""
                    if sums is None:
                        pm, pv2 = ln_sums_alloc()
                        for d in range(NB):
                            ln_sums_step(pm, pv2, src[d], d, pool)
                    else:
                        pm, pv2 = sums
                    mu = pool.tile([1, TOK], F32R, tag="lnmu")
                    with nc.allow_low_precision(reason="f32r mu row"):
                        nc.vector.tensor_scalar_mul(mu[:], pm, 1.0 / D)
                    musq = pool.tile([1, TOK], F32, tag="lnmusq")
                    nc.vector.tensor_mul(musq[:], _f(mu[:]), _f(mu[:]))
                    var = pool.tile([1, TOK], F32, tag="lnvar")
                    nc.vector.scalar_tensor_tensor(
                        var[:], pv2, 1.0 / D, musq[:],
                        op0=AL.mult, op1=AL.subtract)
                    std = pool.tile([1, TOK], F32, tag="lnstd")
                    nc.scalar.activation(std[:], var[:], AF.Sqrt,
                                         bias=eps_t[:])
                    rsd = pool.tile([1, TOK], F32R, tag="lnrsd")
                    with nc.allow_low_precision(reason="f32r rsd row"):
                        nc.vector.reciprocal(rsd[:], std[:])
                    mu_r = pool.tile([P, TOK], F32, tag="lnmur")
                    rsd_r = pool.tile([P, TOK], F32, tag="lnrsdr")
                    nc.gpsimd.partition_broadcast(mu_r[:], _f(mu[:]))
                    nc.gpsimd.partition_broadcast(rsd_r[:], _f(rsd[:]))
                    for d in range(NB):
                        # Pool is ~3x slower than DVE on [P,TOK] f32: give it
                        # only 2 of 8 blocks so neither engine serializes.
                        eng = nc.gpsimd if d in (3, 7) else nc.vector
                        t1 = pool.tile([P, TOK], F32, tag="lnt1", bufs=4)
                        eng.tensor_tensor(out=t1[:], in0=_f(src[d][:]),
                                          in1=mu_r[:], op=AL.subtract)
                        t2 = pool.tile([P, TOK], F32, tag="lnt2", bufs=4)
                        eng.tensor_tensor(out=t2[:], in0=t1[:], in1=rsd_r[:],
                                          op=AL.mult)
                        # gain folds into ACT's per-partition scale
                        nc.scalar.activation(out_f32[d][:], t2[:], AF.Identity,
                                             bias=b_ap[:, d:d + 1],
                                             scale=g_ap[:, d:d + 1])
                        if out_b16 is not None:
                            # bf16 path straight from t2 on DVE (one fused
                            # scale+bias op) — no ACT-output round trip.
                            nc.vector.tensor_scalar(
                                out=out_b16[d][:], in0=t2[:],
                                scalar1=g_ap[:, d:d + 1],
                                scalar2=b_ap[:, d:d + 1],
                                op0=AL.mult, op1=AL.add)
                        if post_d is not None:
                            post_d(d)

                def attn_kv(tc_, pre, kvT, pool, pending_dmas=(),
                            mid_emit=None):
                    """K projections + direct-layout V for all heads.
                    Returns (k_sb[hp] bf16 [P,L], vn[kb] bf16 [P,H,65]).
                    K runs first (scores need it sooner); both K and V^T
                    work in 512-wide halves on the shared "ps" PSUM ring so
                    drain copies double-buffer against the next matmul.
                    DMA issue order: wk0 before the bulky wv transfer so the
                    first K matmul is not stuck behind it in the DGE queue;
                    pending_dmas (input tails) issue in between."""
                    k_sb = []
                    vn = []
                    with tc_.tile_pool(name=f"{pre}_wv", bufs=1) as wvp:
                        wv_t = wvp.tile([P, NB, D], BF16, tag="wvt", bufs=1)
                        for hp in range(NB):
                            wk_t = pool.tile([P, NB, P], BF16, tag="wkt",
                                             bufs=3)
                            nc.sync.dma_start(out=wk_t[:],
                                              in_=dw[f"{pre}_wk"][hp])
                            if hp == 0:
                                for fn in pending_dmas:
                                    fn()
                            if hp == 4:
                                nc.sync.dma_start(
                                    out=wv_t[:], in_=dw[f"{pre}_wv"][:, :, :])
                            k = pool.tile([P, L], BF16, tag=f"ks{hp}",
                                          name=f"ks{hp}", bufs=1)
                            # weight-stationary: load wk block once, stream
                            # both 512-token halves into two PSUM banks.
                            pkh = [psum.tile([P, 512], F32, tag="ps",
                                             name=f"pkh{h}", bufs=3)
                                   for h in range(2)]
                            for t in range(NB):
                                for half in range(2):
                                    sl = slice(half * 512, (half + 1) * 512)
                                    nc.tensor.matmul(pkh[half][:],
                                                     wk_t[:, t, :],
                                                     kvT[t][:, sl],
                                                     start=(t == 0),
                                                     stop=(t == NB - 1))
                            for half in range(2):
                                sl = slice(half * 512, (half + 1) * 512)
                                nc.scalar.copy(out=k[:, sl], in_=pkh[half][:])
                            k_sb.append(k)
                        if mid_emit is not None:
                            mid_emit()
                        for kb in range(NB):
                            v = pool.tile([P, H, 65], BF16, tag=f"vn{kb}",
                                          name=f"vn{kb}", bufs=1)
                            nc.gpsimd.memset(v[:, :, 64:65], 1.0)
                            pvh = [psum.tile([P, 512], F32, tag="ps",
                                             name=f"pvh{h}", bufs=3)
                                   for h in range(2)]
                            for t in range(NB):
                                for half in range(2):
                                    sl = slice(half * 512, (half + 1) * 512)
                                    nc.tensor.matmul(
                                        pvh[half][:],
                                        kvT[t][:, kb * P:(kb + 1) * P],
                                        wv_t[:, t, sl],
                                        start=(t == 0), stop=(t == NB - 1))
                            for half in range(2):
                                nc.scalar.copy(
                                    out=v[:, half * 8:(half + 1) * 8, 0:64],
                                    in_=pvh[half][:].rearrange(
                                        "p (h c) -> p h c", c=64))
                            vn.append(v)
                    return k_sb, vn

                def attn_q(pre, q_src, k_sb, vn, causal, out_tiles, resid,
                           pool, post_db=None):
                    """Q proj + scores + softmax + AV + O proj (+resid)."""
                    bq = bias_sb[f"{pre}_bq"]
                    bo = bias_sb[f"{pre}_bo"]
                    AO = [pool.tile([P, TOK], BF16, tag=f"ao{hp}",
                                    name=f"ao{hp}", bufs=1)
                          for hp in range(NB)]
                    wo_pre = {}
                    for db in range(3):
                        w = pool.tile([P, NB, P], BF16, tag="wot", bufs=3)
                        nc.sync.dma_start(out=w[:], in_=dw[f"{pre}_wo"][db])
                        wo_pre[db] = w
                    for hp in range(NB):
                        wq_t = pool.tile([P, NB, P], BF16, tag="wqt", bufs=3)
                        nc.sync.dma_start(out=wq_t[:], in_=dw[f"{pre}_wq"][hp])
                        pq = psum.tile([P, TOK], F32, tag="pgen", name="pq",
                                       bufs=1)
                        for t in range(NB):
                            nc.tensor.matmul(pq[:], wq_t[:, t, :],
                                             q_src[t][:],
                                             start=(t == 0), stop=(t == NB - 1))
                        q_sb = pool.tile([P, TOK], BF16, tag="qsb", bufs=2)
                        if causal:
                            nc.scalar.activation(q_sb[:], pq[:], AF.Identity,
                                                 bias=bq[:, hp:hp + 1],
                                                 scale=1.0)
                        else:
                            nc.vector.tensor_scalar_add(q_sb[:], pq[:],
                                                        bq[:, hp:hp + 1])

                        hsls = (slice(0, 64), slice(64, 128))
                        eb_ts = []
                        if causal:
                            for hh in range(2):
                                eb_t = pool.tile([P, 1280], BF16, tag="ebt",
                                                 bufs=3)
                                nc.sync.dma_start(out=eb_t[:],
                                                  in_=d_eb[2 * hp + hh])
                                eb_ts.append(eb_t)
                        # interleave the two heads of this partition block so
                        # PE always has a score matmul ready while ACT exps.
                        pts = ([], [])
                        for kb in range(NB):
                            smin = kb // 2 if causal else 0
                            n = TOK - smin * P
                            ns = SLOTS - smin
                            for hh in range(2):
                                ps = psum.tile([P, TOK], F32, tag="ps",
                                               name="ps", bufs=3)
                                nc.tensor.matmul(
                                    ps[:, 0:n],
                                    k_sb[hp][hsls[hh], kb * P:(kb + 1) * P],
                                    q_sb[hsls[hh], smin * P:TOK],
                                    start=True, stop=True)
                                pt = pool.tile([P, TOK], BF16, tag="pt",
                                               bufs=8)
                                nc.scalar.activation(pt[:, 0:n], ps[:, 0:n],
                                                     AF.Exp, scale=0.125)
                                if causal:
                                    w0 = 256 * smin - 128 * kb + 128
                                    ebv = eb_ts[hh][:, w0:w0 + ns * 256] \
                                        .rearrange("p (s c) -> p s c",
                                                   c=256)[:, :, 0:P]
                                    nc.vector.tensor_tensor(
                                        out=pt[:, 0:n].rearrange(
                                            "p (s c) -> p s c", c=P),
                                        in0=pt[:, 0:n].rearrange(
                                            "p (s c) -> p s c", c=P),
                                        in1=ebv, op=AL.mult)
                                pts[hh].append(pt)
                        if dbg and pre == "sa" and hp == 0:
                            nc.sync.dma_start(out=d_dbg["vn"][:, :, :],
                                              in_=vn[0][:])
                            for kb_ in range(2):
                                nc.sync.dma_start(
                                    out=d_dbg["pt"][kb_ * P:(kb_ + 1) * P, :],
                                    in_=pts[0][kb_][:])
                        for hh in range(2):
                            h = 2 * hp + hh
                            pav = psum.tile([65, TOK], F32, tag="pav",
                                            name="pav", bufs=2)
                            for kb in range(NB):
                                smin = kb // 2 if causal else 0
                                n = TOK - smin * P
                                nc.tensor.matmul(
                                    pav[:, smin * P:TOK], vn[kb][:, h, :],
                                    pts[hh][kb][:, 0:n],
                                    start=(kb == 0), stop=(kb == NB - 1))
                            rec = pool.tile([1, TOK], F32R, tag="rec",
                                            bufs=2)
                            with nc.allow_low_precision(reason="recip row"):
                                nc.vector.reciprocal(rec[:], pav[64:65, :])
                            rrep = pool.tile([64, TOK], F32, tag="rrep",
                                             bufs=1)
                            nc.gpsimd.partition_broadcast(rrep[:],
                                                          _f(rec[:]))
                            nc.vector.tensor_tensor(
                                out=AO[hp][hsls[hh], :], in0=pav[0:64, :],
                                in1=rrep[:], op=AL.mult)

                    if dbg and pre == "sa":
                        for hp in range(NB):
                            nc.sync.dma_start(
                                out=d_dbg["ao"][hp * P:(hp + 1) * P, :],
                                in_=AO[hp][:])
                    for db in range(NB):
                        if db in wo_pre:
                            wo_t = wo_pre.pop(db)
                        else:
                            wo_t = pool.tile([P, NB, P], BF16, tag="wot",
                                             bufs=3)
                            nc.sync.dma_start(out=wo_t[:],
                                              in_=dw[f"{pre}_wo"][db])
                        po = psum.tile([P, TOK], F32, tag="ps", name="po",
                                       bufs=3)
                        for hp in range(NB):
                            nc.tensor.matmul(po[:], wo_t[:, hp, :],
                                             AO[hp][:],
                                             start=(hp == 0),
                                             stop=(hp == NB - 1))
                        nc.vector.scalar_tensor_tensor(
                            out_tiles[db][:], po[:], bo[:, db:db + 1],
                            _f(resid[db][:]), op0=AL.add, op1=AL.add)
                        if post_db is not None and db > 0:
                            post_db(db - 1, out_tiles[db - 1])
                    if post_db is not None:
                        post_db(NB - 1, out_tiles[NB - 1])

                with tc.tile_pool(name="xs", bufs=1) as xsp:
                    x1 = [xsp.tile([P, TOK], F32R, tag=f"x1_{d}",
                                   name=f"x1_{d}") for d in range(NB)]
                    x1b = [xsp.tile([P, TOK], BF16, tag=f"x1b_{d}",
                                    name=f"x1b_{d}") for d in range(NB)]
                    x2 = [xsp.tile([P, TOK], F32R, tag=f"x2_{d}",
                                   name=f"x2_{d}") for d in range(NB)]
                    x2b = [xsp.tile([P, TOK], BF16, tag=f"x2b_{d}",
                                    name=f"x2b_{d}") for d in range(NB)]
                    mm = [xsp.tile([P, L], BF16, tag=f"mm{d}",
                                   name=f"mm{d}") for d in range(NB)]

                    # ---------------- self-attention ----------------
                    with tc.tile_pool(name="sa", bufs=1) as sa_pool:
                        xf = [sa_pool.tile([P, L], BF16, tag=f"xf{d}",
                                           name=f"xf{d}") for d in range(NB)]
                        for d in range(2):
                            nc.sync.dma_start(
                                out=xf[d][:, 0:512],
                                in_=d_xf[d * P:(d + 1) * P, 0:512])
                        xo = [sa_pool.tile([P, TOK], BF16, tag=f"xo{d}",
                                           name=f"xo{d}") for d in range(NB)]
                        xo32 = [sa_pool.tile([P, TOK], F32, tag=f"xo32{d}",
                                             name=f"xo32{d}")
                                for d in range(NB)]

                        def _xf_dma(d, half):
                            sl = slice(half * 512, (half + 1) * 512)
                            return lambda: nc.sync.dma_start(
                                out=xf[d][:, sl],
                                in_=d_xf[d * P:(d + 1) * P, sl])

                        ln1_sums = ln_sums_alloc()

                        def _ln1_step(db, tile_):
                            ln_sums_step(ln1_sums[0], ln1_sums[1], tile_,
                                         db, sa_pool)

                        pending = [_xf_dma(d, 0) for d in range(2, NB)] + \
                            [_xf_dma(d, 1) for d in range(NB)]
                        k_sb, vn = attn_kv(tc, "sa", xf, sa_pool, pending)
                        for d in range(NB):
                            nc.sync.dma_start(
                                out=xo[d][:], in_=d_xo[d * P:(d + 1) * P, :])
                            nc.sync.dma_start(
                                out=xo32[d][:],
                                in_=d_xo32[d * P:(d + 1) * P, :])
                        for d in range(NB):
                            nc.sync.dma_start(
                                out=mm[d][:], in_=d_mm[d * P:(d + 1) * P, :])
                        issue_small_dmas()
                        attn_q("sa", xo, k_sb, vn, True, x1, xo32,
                               sa_pool, post_db=_ln1_step)
                        if dbg:
                            for d in range(NB):
                                nc.sync.dma_start(
                                    out=d_dbg["sa_pre"][d * P:(d + 1) * P, :],
                                    in_=_f(x1[d][:]))

                    # ---------------- cross-attention ----------------
                    # CA K/V first: PE chews mem-dependent matmuls while
                    # DVE/ACT run LN1.
                    with tc.tile_pool(name="ca", bufs=1) as ca_pool:
                        def _ln1_finish():
                            layernorm_T(x1, ln_sb["g1"][:], ln_sb["b1"][:],
                                        x1, x1b, ca_pool, sums=ln1_sums)

                        ck_sb, cvn = attn_kv(tc, "ca", mm, ca_pool,
                                             mid_emit=_ln1_finish)
                        if dbg:
                            for d in range(NB):
                                nc.sync.dma_start(
                                    out=d_dbg["x1"][d * P:(d + 1) * P, :],
                                    in_=_f(x1[d][:]))
                        ln2_sums = ln_sums_alloc()

                        def _ln2_step(db, tile_):
                            ln_sums_step(ln2_sums[0], ln2_sums[1], tile_,
                                         db, ca_pool)

                        attn_q("ca", x1b, ck_sb, cvn, False, x2, x1, ca_pool,
                               post_db=_ln2_step)
                        if dbg:
                            for d in range(NB):
                                nc.sync.dma_start(
                                    out=d_dbg["x2_pre"][d * P:(d + 1) * P, :],
                                    in_=_f(x2[d][:]))

                    # ---------------- FFN ----------------
                    with tc.tile_pool(name="ff", bufs=1) as ff_pool:
                        w1_pre = {}
                        for ff in range(3):
                            w = ff_pool.tile([P, NB, P], BF16, tag="w1t",
                                             bufs=3)
                            nc.sync.dma_start(out=w[:], in_=d_fc1[ff])
                            w1_pre[ff] = w
                        layernorm_T(x2, ln_sb["g2"][:], ln_sb["b2"][:],
                                    x2, x2b, ff_pool, sums=ln2_sums)
                        if dbg:
                            for d in range(NB):
                                nc.sync.dma_start(
                                    out=d_dbg["x2"][d * P:(d + 1) * P, :],
                                    in_=_f(x2[d][:]))
                        ht = []
                        w2_pre = {}
                        for ff in range(NF):
                            if ff in w1_pre:
                                w1 = w1_pre.pop(ff)
                            else:
                                w1 = ff_pool.tile([P, NB, P], BF16, tag="w1t",
                                                  bufs=3)
                                nc.sync.dma_start(out=w1[:], in_=d_fc1[ff])
                            if ff in (16, 24):
                                db_ = (ff - 16) // 8
                                w = ff_pool.tile([P, NF, P], BF16, tag="w2t",
                                                 bufs=2)
                                nc.sync.dma_start(out=w[:], in_=d_fc2[db_])
                                w2_pre[db_] = w
                            pf = psum.tile([P, TOK], F32, tag="ps",
                                           name="pf", bufs=3)
                            for t in range(NB):
                                nc.tensor.matmul(pf[:], w1[:, t, :],
                                                 x2b[t][:],
                                                 start=(t == 0),
                                                 stop=(t == NB - 1))
                            h = ff_pool.tile([P, TOK], BF16, tag=f"ht{ff}",
                                             name=f"ht{ff}")
                            nc.scalar.activation(h[:], pf[:], AF.Relu,
                                                 bias=fc1b[:, ff:ff + 1],
                                                 scale=1.0)
                            ht.append(h)
                        x3 = [ff_pool.tile([P, TOK], F32R, tag=f"x3_{d}",
                                           name=f"x3_{d}")
                              for d in range(NB)]
                        ln3_sums = ln_sums_alloc()
                        for db in range(NB):
                            if db in w2_pre:
                                w2 = w2_pre.pop(db)
                            else:
                                w2 = ff_pool.tile([P, NF, P], BF16, tag="w2t",
                                                  bufs=2)
                                nc.sync.dma_start(out=w2[:], in_=d_fc2[db])
                            pf2 = psum.tile([P, TOK], F32, tag="ps",
                                            name="pf2", bufs=3)
                            for t in range(NF):
                                nc.tensor.matmul(pf2[:], w2[:, t, :],
                                                 ht[t][:],
                                                 start=(t == 0),
                                                 stop=(t == NF - 1))
                            nc.vector.scalar_tensor_tensor(
                                x3[db][:], pf2[:], fc2b[:, db:db + 1],
                                _f(x2[db][:]), op0=AL.add, op1=AL.add)
                            if db > 0:
                                ln_sums_step(ln3_sums[0], ln3_sums[1],
                                             x3[db - 1], db - 1, ff_pool)
                        ln_sums_step(ln3_sums[0], ln3_sums[1], x3[NB - 1],
                                     NB - 1, ff_pool)
                        x3ln = [ff_pool.tile([P, TOK], F32, tag=f"x3l{d}",
                                             name=f"x3l{d}")
                                for d in range(NB)]

                        def _out_dma(d):
                            nc.sync.dma_start(
                                out=d_out[d * P:(d + 1) * P, :],
                                in_=x3ln[d][:])

                        layernorm_T(x3, ln_sb["g3"][:], ln_sb["b3"][:],
                                    x3ln, None, ff_pool, sums=ln3_sums,
                                    post_d=_out_dma)

    if STRIP_LDW:
        _strip_redundant_ldweights(nc)
    nc.finalize()
    return nc


@functools.lru_cache(maxsize=4)
def _get_nc(reps=1, dbg=False):
    return _build_nc(reps, dbg)


def _rel_bucket_np(v):
    """T5 causal bucket for relative distance v = q - k (>= 0)."""
    n = np.maximum(v, 0)
    max_exact = NUM_BUCKETS // 2
    nf = np.maximum(n.astype(np.float32), 1.0)
    val_large = max_exact + (
        np.log(nf / max_exact) / math.log(MAX_DISTANCE / max_exact)
        * (NUM_BUCKETS - max_exact)
    ).astype(np.int32)
    val_large = np.minimum(val_large, NUM_BUCKETS - 1)
    return np.where(n < max_exact, n, val_large).astype(np.int32)


def _build_eb(rel_emb, g):
    """EB[h, i, w] = exp(band((w - 128 + 128 g) - i)); 0 where q < k."""
    v = (np.arange(1280)[None, :] - 128 + 128 * g) - np.arange(P)[:, None]
    bucket = _rel_bucket_np(v)                      # [128, 1280]
    band = np.exp(rel_emb[bucket])                  # [128, 1280, 16]
    band = np.transpose(band, (2, 0, 1)).copy()     # [16, 128, 1280]
    band[:, v < 0] = 0.0
    return band.astype(NPBF)


def _rearr_bias(b):
    return np.ascontiguousarray(np.asarray(b).reshape(-1, P).T, np.float32)


def _tile4(w):
    kb, mb = w.shape[0] // P, w.shape[1] // P
    return np.ascontiguousarray(
        w.reshape(kb, P, mb, P).transpose(2, 1, 0, 3).astype(NPBF))


def _make_in_maps(inp):
    x = np.asarray(inp["x"], np.float32)
    mem = np.asarray(inp["mem"], np.float32)
    rel_emb = np.asarray(inp["rel_emb"], np.float32)

    shared = {}
    for k in ("sa_wq", "sa_wk", "sa_wo", "ca_wq", "ca_wk", "ca_wo",
              "fc1_w", "fc2_w"):
        shared[k] = _tile4(np.asarray(inp[k], np.float32))
    for pre in ("sa", "ca"):
        wv = np.asarray(inp[f"{pre}_wv"], np.float32)
        shared[f"{pre}_wv"] = np.ascontiguousarray(
            wv.reshape(NB, P, D).transpose(1, 0, 2).astype(NPBF))
        bo_eff = np.asarray(inp[f"{pre}_bo"], np.float32) + \
            np.asarray(inp[f"{pre}_bv"], np.float32) @ \
            np.asarray(inp[f"{pre}_wo"], np.float32)
        shared[f"{pre}_bo"] = _rearr_bias(bo_eff)
        shared[f"{pre}_bq"] = _rearr_bias(inp[f"{pre}_bq"])
    for k in ("fc1_b", "fc2_b", "ln1_g", "ln1_b", "ln2_g", "ln2_b",
              "ln3_g", "ln3_b"):
        shared[k] = _rearr_bias(inp[k])
    eb = [_build_eb(rel_emb, g) for g in range(2)]

    in_maps = []
    for c in range(8):
        b, g = c // 2, c % 2
        rows = np.concatenate(
            [x[b, (2 * s + g) * P:(2 * s + g + 1) * P] for s in range(SLOTS)])
        m = dict(shared)
        m["xoT"] = np.ascontiguousarray(rows.T.astype(NPBF))
        m["xoT32"] = np.ascontiguousarray(rows.T, np.float32)
        m["xfT"] = np.ascontiguousarray(x[b].T.astype(NPBF))
        m["memT"] = np.ascontiguousarray(mem[b].T.astype(NPBF))
        m["eb"] = eb[g]
        in_maps.append(m)
    return in_maps


def _unshard(results):
    out = np.empty((B, L, D), np.float32)
    for c in range(8):
        b, g = c // 2, c % 2
        oc = results[c]["out_ownT"]        # [D, TOK]
        for s in range(SLOTS):
            out[b, (2 * s + g) * P:(2 * s + g + 1) * P] = \
                oc[:, s * P:(s + 1) * P].T
    return out


def kernel(x, mem, tgt_mask, mem_mask,
           sa_wq, sa_bq, sa_wk, sa_bk, sa_wv, sa_bv, sa_wo, sa_bo, rel_emb,
           ca_wq, ca_bq, ca_wk, ca_bk, ca_wv, ca_bv, ca_wo, ca_bo,
           fc1_w, fc1_b, fc2_w, fc2_b,
           ln1_g, ln1_b, ln2_g, ln2_b, ln3_g, ln3_b, _trace=False):
    nc = _get_nc()
    in_maps = _make_in_maps(dict(
        x=x, mem=mem, rel_emb=rel_emb,
        sa_wq=sa_wq, sa_wk=sa_wk, sa_wv=sa_wv, sa_wo=sa_wo,
        sa_bq=sa_bq, sa_bk=sa_bk, sa_bv=sa_bv, sa_bo=sa_bo,
        ca_wq=ca_wq, ca_wk=ca_wk, ca_wv=ca_wv, ca_wo=ca_wo,
        ca_bq=ca_bq, ca_bk=ca_bk, ca_bv=ca_bv, ca_bo=ca_bo,
        fc1_w=fc1_w, fc1_b=fc1_b, fc2_w=fc2_w, fc2_b=fc2_b,
        ln1_g=ln1_g, ln1_b=ln1_b, ln2_g=ln2_g, ln2_b=ln2_b,
        ln3_g=ln3_g, ln3_b=ln3_b))

    res = run_bass_kernel_spmd(nc, in_maps, list(range(8)), trace=_trace)
    out = _unshard(res.results)
    kernel.last_exec_time_ns = res.exec_time_ns
    return out

